# revision 1
# baseline (speedup 1.0000x reference)
"""CrossNet kernel for Trainium2 (8 NeuronCores, pure data parallel over batch).

Math: reference computes, for i in 0..2:
    s_i = x_k @ w_i          (per-row dot)
    x_k = x * s_i + b_i + x_k
and returns the three intermediate x_k.

Flattened (by induction):  x_k = x * S_k + B_k + x, with
    S_{k+1} = S_k + s_k,  B_k = cumsum(b)[k-1],
    s_k = (1 + S_k) * c_k + d_k,  c_k = x @ w_k,  d_k = B_k @ w_k.
So the device kernel only needs: 3 dots of x with w_j (fused DVE
multiply+reduce), a tiny per-row scalar recurrence producing
t_i = 1 + S_{i+1}, and out_i = x * t_i + cumb_i (TensorE matmuls with a
diag(t) stationary + ones-broadcast of cumb, or a fused DVE op).
"""

import numpy as np

B, N, ORDER, NCORES = 4096, 4096, 3, 8
ROWS = B // NCORES  # 512 rows per core
P = 128
NT = ROWS // P  # 4 partition-tiles per core

# Which outputs are computed on VectorE via fused scalar_tensor_tensor
# (needs a cumb broadcast tile); the rest go through TensorE+PSUM+ScalarE.
import os

V_OUTS = tuple(
    int(c) for c in os.environ.get("CK_V_OUTS", "012") if c in "012"
)
# outputs routed ScalarE-mult + GPSIMD-add (also need a cumb broadcast tile)
G_OUTS = tuple(
    int(c) for c in os.environ.get("CK_G_OUTS", "") if c in "012"
)
USE_TTR = os.environ.get("CK_USE_TTR", "1") == "1"  # phase A fused op vs mul+reduce
STT_REAL_OUT = os.environ.get("CK_STT_REAL_OUT", "1") == "1"

_prog_cache = {}


def _build_program():
    from contextlib import ExitStack

    import concourse.bacc as bacc
    import concourse.mybir as mybir
    import concourse.tile as tile

    f32 = mybir.dt.float32
    Alu = mybir.AluOpType

    nc = bacc.Bacc("TRN2")
    xs = nc.dram_tensor("xs", [ROWS, N], f32, kind="ExternalInput")
    wr = nc.dram_tensor("wr", [ORDER, N], f32, kind="ExternalInput")
    cb = nc.dram_tensor("cb", [ORDER, N], f32, kind="ExternalInput")
    dd = nc.dram_tensor("dd", [P, ORDER], f32, kind="ExternalInput")
    eye = nc.dram_tensor("eye", [P, P], f32, kind="ExternalInput")
    out = nc.dram_tensor("out", [ORDER, ROWS, N], f32, kind="ExternalOutput")

    HALF = 2048  # psum tile free size (4 banks)

    with ExitStack() as ctx:
        tc = ctx.enter_context(tile.TileContext(nc))
        consts = ctx.enter_context(tc.tile_pool(name="consts", bufs=1))
        xpool = ctx.enter_context(tc.tile_pool(name="xpool", bufs=int(os.environ.get("CK_XBUFS", "2"))))
        small = ctx.enter_context(tc.tile_pool(name="small", bufs=2))
        opool = ctx.enter_context(tc.tile_pool(name="opool", bufs=int(os.environ.get("CK_OBUFS", "2"))))
        psum = ctx.enter_context(tc.tile_pool(name="psum", bufs=2, space="PSUM"))
        scratchpool = ctx.enter_context(tc.tile_pool(name="scratch", bufs=1))

        # w and cumb rows packed at partition bases {0, 32, 64} — the only
        # bases matmul operands may start at. One all-ones tile serves as the
        # broadcast lhsT at any of those bases.
        # staging rows live in the output pool's slots: they are fully
        # consumed by the setup broadcasts before the first ob is needed,
        # and must NOT occupy x-tile slots (that would stall the x loads).
        wpack = opool.tile([2 * 32 + 1, N], f32, tag="ob")
        cpack = opool.tile([2 * 32 + 1, N], f32, tag="ob")
        for j in range(ORDER):
            nc.sync.dma_start(out=wpack[32 * j : 32 * j + 1, :], in_=wr[j : j + 1, :])
            nc.sync.dma_start(out=cpack[32 * j : 32 * j + 1, :], in_=cb[j : j + 1, :])
        dd_t = consts.tile([P, ORDER], f32, tag="dd")
        nc.sync.dma_start(out=dd_t, in_=dd[:, :])
        eye_t = consts.tile([P, P], f32, tag="eye")
        nc.sync.dma_start(out=eye_t, in_=eye[:, :])
        opack = consts.tile([2 * 32 + 1, P], f32, tag="opack")
        nc.vector.memset(opack, 1.0)

        def row_of(pack, j):
            return pack[32 * j : 32 * j + 1, :]

        def one_row(j):
            return opack[32 * j : 32 * j + 1, :]

        def pe_broadcast(dst, pack, j):
            # dst[128, N] = broadcast of pack row j via ones-matmul.
            for h in range(N // HALF):
                pt = psum.tile([P, HALF], f32, tag="ps")
                for q in range(HALF // 512):
                    sl = slice(h * HALF + q * 512, h * HALF + (q + 1) * 512)
                    nc.tensor.matmul(
                        pt[:, q * 512 : (q + 1) * 512],
                        lhsT=one_row(j),
                        rhs=row_of(pack, j)[:, sl],
                        start=True,
                        stop=True,
                    )
                nc.scalar.copy(dst[:, h * HALF : (h + 1) * HALF], pt)

        wb = [
            consts.tile([P, N], f32, tag=f"wb{j}", name=f"wb{j}")
            for j in range(ORDER)
        ]
        cbb = {
            i: consts.tile([P, N], f32, tag=f"cbb{i}", name=f"cbb{i}")
            for i in sorted(set(V_OUTS) | set(G_OUTS))
        }
        # interleave so the tiles phase A/C need first are built first
        build = [(wb[j], wpack, j) for j in range(ORDER)]
        build += [(cbb[i], cpack, i) for i in sorted(cbb)]
        for dst, pack, j in build:
            pe_broadcast(dst, pack, j)

        for k in range(NT):
            rows = slice(k * P, (k + 1) * P)
            x_t = xpool.tile([P, N], f32, tag="x")
            nc.sync.dma_start(out=x_t, in_=xs[rows, :])

            # phase A: c_j = sum_n x * w_j  (fused multiply+reduce on DVE)
            scratch = scratchpool.tile([P, HALF], f32, tag="scr")
            cs = []
            for j in range(ORDER):
                cj = small.tile([P, 1], f32, tag=f"c{j}")
                parts = []
                for h in range(N // HALF):
                    sl = slice(h * HALF, (h + 1) * HALF)
                    cp = small.tile([P, 1], f32, tag=f"cp{j}{h}")
                    nc.vector.scalar_tensor_tensor(
                        out=scratch,
                        in0=x_t[:, sl],
                        scalar=1.0,
                        in1=wb[j][:, sl],
                        op0=Alu.mult,
                        op1=Alu.mult,
                        accum_out=cp,
                    )
                    parts.append(cp)
                nc.vector.tensor_add(cj, parts[0], parts[1])
                cs.append(cj)

            # recurrence: t_1 = 1 + c_0 ; s_i = t_i * c_i + d_i ; t_{i+1} = t_i + s_i
            ts = []
            t1 = small.tile([P, 1], f32, tag="t0")
            nc.vector.tensor_scalar_add(t1, cs[0], 1.0)
            ts.append(t1)
            for i in range(1, ORDER):
                si = small.tile([P, 1], f32, tag=f"s{i}")
                nc.vector.tensor_scalar(
                    out=si,
                    in0=cs[i],
                    scalar1=ts[i - 1],
                    scalar2=dd_t[:, i : i + 1],
                    op0=Alu.mult,
                    op1=Alu.add,
                )
                ti = small.tile([P, 1], f32, tag=f"t{i}")
                nc.vector.tensor_add(ti, ts[i - 1], si)
                ts.append(ti)

            # phase C: out_i = x * t_i + cumb_i
            for i in range(ORDER):
                ob = opool.tile([P, N], f32, tag="ob")
                if i in V_OUTS:
                    for h in range(N // HALF):
                        sl = slice(h * HALF, (h + 1) * HALF)
                        nc.vector.scalar_tensor_tensor(
                            out=ob[:, sl],
                            in0=x_t[:, sl],
                            scalar=ts[i],
                            in1=cbb[i][:, sl],
                            op0=Alu.mult,
                            op1=Alu.add,
                        )
                elif i in G_OUTS:
                    # ScalarE: ob = x * t_i ; GPSIMD: ob += cumb_i
                    nc.scalar.mul(ob, x_t, ts[i])
                    nc.gpsimd.tensor_add(ob, ob, cbb[i])
                else:
                    dg = small.tile([P, P], f32, tag=f"dg{i}")
                    nc.vector.tensor_scalar_mul(dg, eye_t, ts[i])
                    for h in range(N // HALF):
                        pt = psum.tile([P, HALF], f32, tag="ps")
                        for q in range(HALF // 512):
                            sl = slice(h * HALF + q * 512, h * HALF + (q + 1) * 512)
                            nc.tensor.matmul(
                                pt[:, q * 512 : (q + 1) * 512],
                                lhsT=one_row(i),
                                rhs=row_of(cpack, i)[:, sl],
                                start=True,
                                stop=False,
                            )
                        for q in range(HALF // 512):
                            sl = slice(h * HALF + q * 512, h * HALF + (q + 1) * 512)
                            nc.tensor.matmul(
                                pt[:, q * 512 : (q + 1) * 512],
                                lhsT=dg,
                                rhs=x_t[:, sl],
                                start=False,
                                stop=True,
                            )
                        nc.scalar.copy(ob[:, h * HALF : (h + 1) * HALF], pt)
                nc.sync.dma_start(out=out[i, rows, :], in_=ob)

    nc.finalize()
    return nc


def _get_program():
    if "nc" not in _prog_cache:
        _prog_cache["nc"] = _build_program()
    return _prog_cache["nc"]


def _prep_inputs(x, w, b):
    x = np.ascontiguousarray(np.asarray(x, dtype=np.float32))
    w_r = np.asarray(w, dtype=np.float32).reshape(ORDER, N)
    b_r = np.asarray(b, dtype=np.float32).reshape(ORDER, N)
    cumb = np.cumsum(b_r, axis=0).astype(np.float32)  # cumb[i] = b_0 + ... + b_i
    d = np.zeros(ORDER, dtype=np.float64)
    for i in range(1, ORDER):
        d[i] = cumb[i - 1].astype(np.float64) @ w_r[i].astype(np.float64)
    dd = np.tile(d.astype(np.float32)[None, :], (P, 1))
    eye = np.eye(P, dtype=np.float32)

    shared = {
        "wr": np.ascontiguousarray(w_r),
        "cb": np.ascontiguousarray(cumb),
        "dd": np.ascontiguousarray(dd),
        "eye": eye,
    }
    in_maps = []
    for c in range(NCORES):
        m = dict(shared)
        m["xs"] = np.ascontiguousarray(x[c * ROWS : (c + 1) * ROWS, :])
        in_maps.append(m)
    return in_maps


def _run(x, w, b, trace=False):
    from concourse.bass_utils import run_bass_kernel_spmd

    nc = _get_program()
    in_maps = _prep_inputs(x, w, b)
    res = run_bass_kernel_spmd(nc, in_maps, core_ids=list(range(NCORES)), trace=trace)
    outs = [np.asarray(r["out"]) for r in res.results]  # each [ORDER, ROWS, N]
    full = np.concatenate(outs, axis=1)  # [ORDER, B, N]
    return tuple(np.ascontiguousarray(full[i]) for i in range(ORDER)), res


def kernel(x, w, b):
    outs, _ = _run(x, w, b, trace=False)
    return outs



# revision 2
# speedup vs baseline: 1.4637x; 1.4637x over previous
"""CrossNet kernel for Trainium2 (8 NeuronCores, pure data parallel over batch).

Math: reference computes, for i in 0..2:
    s_i = x_k @ w_i          (per-row dot)
    x_k = x * s_i + b_i + x_k
and returns the three intermediate x_k.

Flattened (by induction):  x_k = x * S_k + B_k + x, with
    S_{k+1} = S_k + s_k,  B_k = cumsum(b)[k-1],
    s_k = (1 + S_k) * c_k + d_k,  c_k = x @ w_k,  d_k = B_k @ w_k.
So the device kernel only needs: 3 dots of x with w_j (fused DVE
multiply+reduce), a tiny per-row scalar recurrence producing
t_i = 1 + S_{i+1}, and out_i = x * t_i + cumb_i.

v2: fp16 end-to-end on device (tolerance is 2e-2; fp16 costs ~3e-3).
Halves DMA bytes (the binding resource: 16.8 MiB/core vs 33.6), doubles
DVE throughput, and makes the PE broadcast setup 4x faster. Phase C is
split between DVE (fused scalar_tensor_tensor) and PE (diag(t) matmul +
ones x cumb accumulate) to keep every engine under the DMA roofline.
"""

import os

import numpy as np

B, N, ORDER, NCORES = 4096, 4096, 3, 8
ROWS = B // NCORES  # 512 rows per core
P = 128
NT = ROWS // P  # 4 partition-tiles per core

# Which outputs are computed on VectorE via fused scalar_tensor_tensor
# (needs a cumb broadcast tile); the rest go through TensorE+PSUM+ScalarE.
V_OUTS = tuple(
    int(c) for c in os.environ.get("CK_V_OUTS", "01") if c in "012"
)
# DMA queue per output index: 's' = SP (sync) HWDGE, 'a' = Activation HWDGE
OUT_Q = os.environ.get("CK_OUT_Q", "sas")
X_Q = os.environ.get("CK_X_Q", "s")
XBUFS = int(os.environ.get("CK_XBUFS", "3"))
OBUFS = int(os.environ.get("CK_OBUFS", "4"))

_prog_cache = {}


def _build_program():
    from contextlib import ExitStack

    import concourse.bacc as bacc
    import concourse.mybir as mybir
    import concourse.tile as tile

    f32 = mybir.dt.float32
    f16 = mybir.dt.float16
    Alu = mybir.AluOpType

    nc = bacc.Bacc("TRN2")
    xs = nc.dram_tensor("xs", [ROWS, N], f16, kind="ExternalInput")
    wr = nc.dram_tensor("wr", [ORDER, N], f16, kind="ExternalInput")
    cb = nc.dram_tensor("cb", [ORDER, N], f16, kind="ExternalInput")
    dd = nc.dram_tensor("dd", [P, ORDER], f32, kind="ExternalInput")
    eye = nc.dram_tensor("eye", [P, P], f16, kind="ExternalInput")
    out = nc.dram_tensor("out", [ORDER, ROWS, N], f16, kind="ExternalOutput")

    HALF = 2048  # psum tile free size (4 banks at fp32)

    def q_eng(c):
        return nc.scalar if c == "a" else nc.sync

    with ExitStack() as ctx:
        tc = ctx.enter_context(tile.TileContext(nc))
        consts = ctx.enter_context(tc.tile_pool(name="consts", bufs=1))
        xpool = ctx.enter_context(tc.tile_pool(name="xpool", bufs=XBUFS))
        small = ctx.enter_context(tc.tile_pool(name="small", bufs=2))
        opool = ctx.enter_context(tc.tile_pool(name="opool", bufs=OBUFS))
        psum = ctx.enter_context(tc.tile_pool(name="psum", bufs=2, space="PSUM"))
        scratchpool = ctx.enter_context(tc.tile_pool(name="scratch", bufs=1))

        # w and cumb rows packed at partition bases {0, 32, 64} — the only
        # bases matmul operands may start at. One all-ones tile serves as the
        # broadcast lhsT at any of those bases.
        # staging rows live in the output pool's slots: they are fully
        # consumed by the setup broadcasts before the first ob is needed,
        # and must NOT occupy x-tile slots (that would stall the x loads).
        wpack = opool.tile([2 * 32 + 1, N], f16, tag="ob")
        cpack = opool.tile([2 * 32 + 1, N], f16, tag="ob")
        for j in range(ORDER):
            nc.sync.dma_start(out=wpack[32 * j : 32 * j + 1, :], in_=wr[j : j + 1, :])
            nc.sync.dma_start(out=cpack[32 * j : 32 * j + 1, :], in_=cb[j : j + 1, :])
        dd_t = consts.tile([P, ORDER], f32, tag="dd")
        nc.sync.dma_start(out=dd_t, in_=dd[:, :])
        eye_t = consts.tile([P, P], f16, tag="eye")
        nc.sync.dma_start(out=eye_t, in_=eye[:, :])
        opack = consts.tile([2 * 32 + 1, P], f16, tag="opack")
        nc.vector.memset(opack, 1.0)

        def row_of(pack, j):
            return pack[32 * j : 32 * j + 1, :]

        def one_row(j):
            return opack[32 * j : 32 * j + 1, :]

        def pe_broadcast(dst, pack, j):
            # dst[128, N] = broadcast of pack row j via ones-matmul.
            for h in range(N // HALF):
                pt = psum.tile([P, HALF], f32, tag="ps")
                for q in range(HALF // 512):
                    sl = slice(h * HALF + q * 512, h * HALF + (q + 1) * 512)
                    nc.tensor.matmul(
                        pt[:, q * 512 : (q + 1) * 512],
                        lhsT=one_row(j),
                        rhs=row_of(pack, j)[:, sl],
                        start=True,
                        stop=True,
                    )
                nc.scalar.copy(dst[:, h * HALF : (h + 1) * HALF], pt)

        wb = [
            consts.tile([P, N], f16, tag=f"wb{j}", name=f"wb{j}")
            for j in range(ORDER)
        ]
        cbb = {
            i: consts.tile([P, N], f16, tag=f"cbb{i}", name=f"cbb{i}")
            for i in sorted(V_OUTS)
        }
        # interleave so the tiles phase A/C need first are built first
        build = [(wb[j], wpack, j) for j in range(ORDER)]
        build += [(cbb[i], cpack, i) for i in sorted(cbb)]
        for dst, pack, j in build:
            pe_broadcast(dst, pack, j)

        for k in range(NT):
            rows = slice(k * P, (k + 1) * P)
            x_t = xpool.tile([P, N], f16, tag="x")
            q_eng(X_Q).dma_start(out=x_t, in_=xs[rows, :])

            # phase A: c_j = sum_n x * w_j  (fused multiply+reduce on DVE)
            scratch = scratchpool.tile([P, N], f16, tag="scr")
            cs = []
            for j in range(ORDER):
                cj = small.tile([P, 1], f32, tag=f"c{j}")
                nc.vector.scalar_tensor_tensor(
                    out=scratch,
                    in0=x_t,
                    scalar=1.0,
                    in1=wb[j],
                    op0=Alu.mult,
                    op1=Alu.mult,
                    accum_out=cj,
                )
                cs.append(cj)

            # recurrence: t_1 = 1 + c_0 ; s_i = t_i * c_i + d_i ; t_{i+1} = t_i + s_i
            ts = []
            t1 = small.tile([P, 1], f32, tag="t0")
            nc.vector.tensor_scalar_add(t1, cs[0], 1.0)
            ts.append(t1)
            for i in range(1, ORDER):
                si = small.tile([P, 1], f32, tag=f"s{i}")
                nc.vector.tensor_scalar(
                    out=si,
                    in0=cs[i],
                    scalar1=ts[i - 1],
                    scalar2=dd_t[:, i : i + 1],
                    op0=Alu.mult,
                    op1=Alu.add,
                )
                ti = small.tile([P, 1], f32, tag=f"t{i}")
                nc.vector.tensor_add(ti, ts[i - 1], si)
                ts.append(ti)

            # phase C: out_i = x * t_i + cumb_i
            for i in range(ORDER):
                ob = opool.tile([P, N], f16, tag="ob")
                if i in V_OUTS:
                    nc.vector.scalar_tensor_tensor(
                        out=ob,
                        in0=x_t,
                        scalar=ts[i],
                        in1=cbb[i],
                        op0=Alu.mult,
                        op1=Alu.add,
                    )
                else:
                    dg = small.tile([P, P], f16, tag=f"dg{i}")
                    nc.vector.tensor_scalar_mul(dg, eye_t, ts[i])
                    for h in range(N // HALF):
                        pt = psum.tile([P, HALF], f32, tag="ps")
                        for q in range(HALF // 512):
                            sl = slice(h * HALF + q * 512, h * HALF + (q + 1) * 512)
                            nc.tensor.matmul(
                                pt[:, q * 512 : (q + 1) * 512],
                                lhsT=one_row(i),
                                rhs=row_of(cpack, i)[:, sl],
                                start=True,
                                stop=False,
                            )
                        for q in range(HALF // 512):
                            sl = slice(h * HALF + q * 512, h * HALF + (q + 1) * 512)
                            nc.tensor.matmul(
                                pt[:, q * 512 : (q + 1) * 512],
                                lhsT=dg,
                                rhs=x_t[:, sl],
                                start=False,
                                stop=True,
                            )
                        nc.scalar.copy(ob[:, h * HALF : (h + 1) * HALF], pt)
                q_eng(OUT_Q[i]).dma_start(out=out[i, rows, :], in_=ob)

    nc.finalize()
    return nc


def _get_program():
    if "nc" not in _prog_cache:
        _prog_cache["nc"] = _build_program()
    return _prog_cache["nc"]


def _prep_inputs(x, w, b):
    x16 = np.asarray(x, dtype=np.float32).astype(np.float16)
    w_r = np.asarray(w, dtype=np.float32).reshape(ORDER, N).astype(np.float16)
    b_r = np.asarray(b, dtype=np.float32).reshape(ORDER, N)
    cumb = np.cumsum(b_r, axis=0).astype(np.float16)  # cumb[i] = b_0 + ... + b_i
    d = np.zeros(ORDER, dtype=np.float64)
    for i in range(1, ORDER):
        d[i] = cumb[i - 1].astype(np.float64) @ w_r[i].astype(np.float64)
    dd = np.tile(d.astype(np.float32)[None, :], (P, 1))
    eye = np.eye(P, dtype=np.float16)

    shared = {
        "wr": np.ascontiguousarray(w_r),
        "cb": np.ascontiguousarray(cumb),
        "dd": np.ascontiguousarray(dd),
        "eye": eye,
    }
    in_maps = []
    for c in range(NCORES):
        m = dict(shared)
        m["xs"] = np.ascontiguousarray(x16[c * ROWS : (c + 1) * ROWS, :])
        in_maps.append(m)
    return in_maps


def _run(x, w, b, trace=False):
    from concourse.bass_utils import run_bass_kernel_spmd

    nc = _get_program()
    in_maps = _prep_inputs(x, w, b)
    res = run_bass_kernel_spmd(nc, in_maps, core_ids=list(range(NCORES)), trace=trace)
    outs = [np.asarray(r["out"]) for r in res.results]  # each [ORDER, ROWS, N] f16
    full = np.concatenate(outs, axis=1)  # [ORDER, B, N]
    return (
        tuple(np.ascontiguousarray(full[i]).astype(np.float32) for i in range(ORDER)),
        res,
    )


def kernel(x, w, b):
    outs, _ = _run(x, w, b, trace=False)
    return outs


# revision 3
# speedup vs baseline: 1.4809x; 1.0117x over previous
"""CrossNet kernel for Trainium2 (8 NeuronCores, pure data parallel over batch).

Math: reference computes, for i in 0..2:
    s_i = x_k @ w_i          (per-row dot)
    x_k = x * s_i + b_i + x_k
and returns the three intermediate x_k.

Flattened (by induction):  x_k = x * S_k + B_k + x, with
    S_{k+1} = S_k + s_k,  B_k = cumsum(b)[k-1],
    s_k = (1 + S_k) * c_k + d_k,  c_k = x @ w_k,  d_k = B_k @ w_k.
So the device kernel needs: 3 per-row dots of x with w_j, a tiny scalar
recurrence producing t_i = 1 + S_{i+1}, and out_i = x * t_i + cumb_i.

v3: fp16 end-to-end (tolerance 2e-2; fp16 costs ~5e-4). Engine plan per
measured rates (per [128,4096] fp16 pass): DVE STT+accum 4.4us (dots are
DVE-only -> 53us/core, the wall), DVE tensor_scalar 1.3us / TT 2.3us,
ScalarE ACT ~3.8us, PE matmul ~0.4-0.8us per 512-chunk (pstate), Pool TT
7.9us. Phase C leaves DVE: lanes 'p' = PE diag-matmul+cumb -> PSUM ->
ScalarE copy; 'g' = ScalarE mul + Pool add; 'd' = DVE ts-mul + TT add
(fast-tail for the last tile); 'v' = DVE STT. Broadcast tiles built by
DMA row-replication ('d') or PE ones-matmul + ScalarE copy ('p').
"""

import os

import numpy as np

B, N, ORDER, NCORES = 4096, 4096, 3, 8
ROWS = B // NCORES  # 512 rows per core
P = 128
NT = ROWS // P  # 4 partition-tiles per core

# lane per output index: 'p' PE+ScalarE, 'g' ScalarE+Pool, 'v' DVE STT,
# 'd' DVE tensor_scalar + tensor_tensor
LANES = os.environ.get("CK_LANES", "ppg")
LANES_LAST = os.environ.get("CK_LANES_LAST", "ppd")
# broadcast builder per tile wb0,wb1,wb2,cbb: 'd' DMA replicate, 'p' PE
BCAST = os.environ.get("CK_BCAST", "ddpp")
# DMA queue per output index ('s' sync / 'a' scalar), and for x loads
OUT_Q = os.environ.get("CK_OUT_Q", "sas")
X_Q = os.environ.get("CK_X_Q", "s")
XBUFS = int(os.environ.get("CK_XBUFS", "3"))
OBUFS = int(os.environ.get("CK_OBUFS", "4"))
TBUFS = int(os.environ.get("CK_TBUFS", "2"))

_prog_cache = {}


def _build_program():
    from contextlib import ExitStack

    import concourse.bacc as bacc
    import concourse.mybir as mybir
    import concourse.tile as tile

    f32 = mybir.dt.float32
    f16 = mybir.dt.float16
    Alu = mybir.AluOpType

    nc = bacc.Bacc("TRN2")
    xs = nc.dram_tensor("xs", [ROWS, N], f16, kind="ExternalInput")
    wr = nc.dram_tensor("wr", [ORDER, N], f16, kind="ExternalInput")
    cb = nc.dram_tensor("cb", [ORDER, N], f16, kind="ExternalInput")
    dd = nc.dram_tensor("dd", [P, ORDER], f32, kind="ExternalInput")
    eye = nc.dram_tensor("eye", [P, P], f16, kind="ExternalInput")
    out = nc.dram_tensor("out", [ORDER, ROWS, N], f16, kind="ExternalOutput")

    HALF = 2048  # psum tile free size (4 banks at fp32)

    lanes_by_tile = [LANES] * (NT - 1) + [LANES_LAST]
    need_cbb = sorted(
        {i for ls in lanes_by_tile for i, c in enumerate(ls) if c in "gvd"}
    )

    def q_eng(c):
        return nc.scalar if c == "a" else nc.sync

    with ExitStack() as ctx:
        tc = ctx.enter_context(tile.TileContext(nc))
        consts = ctx.enter_context(tc.tile_pool(name="consts", bufs=1))
        xpool = ctx.enter_context(tc.tile_pool(name="xpool", bufs=XBUFS))
        small = ctx.enter_context(tc.tile_pool(name="small", bufs=2))
        opool = ctx.enter_context(tc.tile_pool(name="opool", bufs=OBUFS))
        tpool = ctx.enter_context(tc.tile_pool(name="tpool", bufs=TBUFS))
        psum = ctx.enter_context(tc.tile_pool(name="psum", bufs=2, space="PSUM"))
        scratchpool = ctx.enter_context(tc.tile_pool(name="scratch", bufs=1))

        # w and cumb rows packed at partition bases {0, 32, 64} — the only
        # bases matmul operands may start at. One all-ones tile serves as the
        # broadcast lhsT at any of those bases. Staged in opool slots (fully
        # consumed by setup before the first ob is needed).
        wpack = opool.tile([2 * 32 + 1, N], f16, tag="ob")
        cpack = opool.tile([2 * 32 + 1, N], f16, tag="ob")
        for j in range(ORDER):
            nc.scalar.dma_start(out=wpack[32 * j : 32 * j + 1, :], in_=wr[j : j + 1, :])
            nc.scalar.dma_start(out=cpack[32 * j : 32 * j + 1, :], in_=cb[j : j + 1, :])
        dd_t = consts.tile([P, ORDER], f32, tag="dd")
        nc.scalar.dma_start(out=dd_t, in_=dd[:, :])
        eye_t = consts.tile([P, P], f16, tag="eye")
        nc.scalar.dma_start(out=eye_t, in_=eye[:, :])
        opack = consts.tile([2 * 32 + 1, P], f16, tag="opack")
        nc.vector.memset(opack, 1.0)

        def row_of(pack, j):
            return pack[32 * j : 32 * j + 1, :]

        def one_row(j):
            return opack[32 * j : 32 * j + 1, :]

        def pe_broadcast(dst, pack, j):
            # dst[128, N] = broadcast of pack row j via ones-matmul.
            for h in range(N // HALF):
                pt = psum.tile([P, HALF], f32, tag="ps")
                for q in range(HALF // 512):
                    sl = slice(h * HALF + q * 512, h * HALF + (q + 1) * 512)
                    nc.tensor.matmul(
                        pt[:, q * 512 : (q + 1) * 512],
                        lhsT=one_row(j),
                        rhs=row_of(pack, j)[:, sl],
                        start=True,
                        stop=True,
                    )
                nc.scalar.copy(dst[:, h * HALF : (h + 1) * HALF], pt)

        wb = [
            consts.tile([P, N], f16, tag=f"wb{j}", name=f"wb{j}")
            for j in range(ORDER)
        ]
        cbb = {
            i: consts.tile([P, N], f16, tag=f"cbb{i}", name=f"cbb{i}")
            for i in need_cbb
        }
        # build broadcast tiles; DMA replication goes on the scalar queue so
        # it overlaps the x loads on the sync queue during lead-in.
        for bi, (dst, src_dram, pack, j) in enumerate(
            [(wb[j], wr, wpack, j) for j in range(ORDER)]
            + [(cbb[i], cb, cpack, i) for i in need_cbb]
        ):
            mode = BCAST[bi] if bi < len(BCAST) else "p"
            if mode == "d":
                nc.scalar.dma_start(
                    out=dst, in_=src_dram[j : j + 1, :].partition_broadcast(P)
                )
            else:
                pe_broadcast(dst, pack, j)

        for k in range(NT):
            lanes = lanes_by_tile[k]
            rows = slice(k * P, (k + 1) * P)
            x_t = xpool.tile([P, N], f16, tag="x")
            q_eng(X_Q).dma_start(out=x_t, in_=xs[rows, :])

            scratch = scratchpool.tile([P, N], f16, tag="scr")
            cs, ts = [], []
            obs = [None] * ORDER
            for j in range(ORDER):
                # dot_j = sum_n x * w_j (fused multiply+reduce on DVE)
                cj = small.tile([P, 1], f32, tag=f"c{j}")
                nc.vector.scalar_tensor_tensor(
                    out=scratch,
                    in0=x_t,
                    scalar=1.0,
                    in1=wb[j],
                    op0=Alu.mult,
                    op1=Alu.mult,
                    accum_out=cj,
                )
                cs.append(cj)
                # recurrence step -> ts[j] available right after dot j
                if j == 0:
                    t1 = small.tile([P, 1], f32, tag="t0")
                    nc.vector.tensor_scalar_add(t1, cs[0], 1.0)
                    ts.append(t1)
                else:
                    si = small.tile([P, 1], f32, tag=f"s{j}")
                    nc.vector.tensor_scalar(
                        out=si,
                        in0=cs[j],
                        scalar1=ts[j - 1],
                        scalar2=dd_t[:, j : j + 1],
                        op0=Alu.mult,
                        op1=Alu.add,
                    )
                    ti = small.tile([P, 1], f32, tag=f"t{j}")
                    nc.vector.tensor_add(ti, ts[j - 1], si)
                    ts.append(ti)

                # phase C for output j (t_j just became available)
                i = j
                ob = opool.tile([P, N], f16, tag="ob")
                lane = lanes[i]
                if lane == "v":
                    nc.vector.scalar_tensor_tensor(
                        out=ob,
                        in0=x_t,
                        scalar=ts[i],
                        in1=cbb[i],
                        op0=Alu.mult,
                        op1=Alu.add,
                    )
                elif lane == "d":
                    tmp = tpool.tile([P, N], f16, tag="tmp")
                    nc.vector.tensor_scalar(
                        out=tmp, in0=x_t, scalar1=ts[i], scalar2=None, op0=Alu.mult
                    )
                    nc.vector.tensor_add(ob, tmp, cbb[i])
                elif lane == "g":
                    tmp = tpool.tile([P, N], f16, tag="tmp")
                    nc.scalar.mul(tmp, x_t, ts[i])
                    nc.gpsimd.tensor_add(ob, tmp, cbb[i])
                else:  # 'p'
                    dg = small.tile([P, P], f16, tag=f"dg{i}")
                    nc.vector.tensor_scalar_mul(dg, eye_t, ts[i])
                    for h in range(N // HALF):
                        pt = psum.tile([P, HALF], f32, tag="ps")
                        for q in range(HALF // 512):
                            sl = slice(h * HALF + q * 512, h * HALF + (q + 1) * 512)
                            nc.tensor.matmul(
                                pt[:, q * 512 : (q + 1) * 512],
                                lhsT=one_row(i),
                                rhs=row_of(cpack, i)[:, sl],
                                start=True,
                                stop=False,
                            )
                        for q in range(HALF // 512):
                            sl = slice(h * HALF + q * 512, h * HALF + (q + 1) * 512)
                            nc.tensor.matmul(
                                pt[:, q * 512 : (q + 1) * 512],
                                lhsT=dg,
                                rhs=x_t[:, sl],
                                start=False,
                                stop=True,
                            )
                        nc.scalar.copy(ob[:, h * HALF : (h + 1) * HALF], pt)
                obs[i] = ob
                q_eng(OUT_Q[i]).dma_start(out=out[i, rows, :], in_=ob)

    nc.finalize()
    return nc


def _get_program():
    if "nc" not in _prog_cache:
        _prog_cache["nc"] = _build_program()
    return _prog_cache["nc"]


def _prep_inputs(x, w, b):
    x16 = np.asarray(x, dtype=np.float32).astype(np.float16)
    w_r = np.asarray(w, dtype=np.float32).reshape(ORDER, N).astype(np.float16)
    b_r = np.asarray(b, dtype=np.float32).reshape(ORDER, N)
    cumb = np.cumsum(b_r, axis=0).astype(np.float16)  # cumb[i] = b_0 + ... + b_i
    d = np.zeros(ORDER, dtype=np.float64)
    for i in range(1, ORDER):
        d[i] = cumb[i - 1].astype(np.float64) @ w_r[i].astype(np.float64)
    dd = np.tile(d.astype(np.float32)[None, :], (P, 1))
    eye = np.eye(P, dtype=np.float16)

    shared = {
        "wr": np.ascontiguousarray(w_r),
        "cb": np.ascontiguousarray(cumb),
        "dd": np.ascontiguousarray(dd),
        "eye": eye,
    }
    in_maps = []
    for c in range(NCORES):
        m = dict(shared)
        m["xs"] = np.ascontiguousarray(x16[c * ROWS : (c + 1) * ROWS, :])
        in_maps.append(m)
    return in_maps


def _run(x, w, b, trace=False):
    from concourse.bass_utils import run_bass_kernel_spmd

    nc = _get_program()
    in_maps = _prep_inputs(x, w, b)
    res = run_bass_kernel_spmd(nc, in_maps, core_ids=list(range(NCORES)), trace=trace)
    outs = [np.asarray(r["out"]) for r in res.results]  # each [ORDER, ROWS, N] f16
    full = np.concatenate(outs, axis=1)  # [ORDER, B, N]
    return (
        tuple(np.ascontiguousarray(full[i]).astype(np.float32) for i in range(ORDER)),
        res,
    )


def kernel(x, w, b):
    outs, _ = _run(x, w, b, trace=False)
    return outs
